# revision 13
# baseline (speedup 1.0000x reference)
"""BoxMultiHeadedAttention Trainium2 kernel.

Self-contained: takes FULL inputs, shards batch 16 -> 8 cores x 2,
runs a Bass/Tile kernel per core via run_bass_kernel_spmd, gathers.

Math (per batch b, head h, query n, key m):
  q,k,v = x @ W + b ; scores^T[m,n] = (k_h q_h^T)/sqrt(64)  (scale folded in Wq)
  geo bias w_g[h,n,m] = relu(geo(box) @ WG_w[h] + WG_b[h])
  p ~ softmax_m(log(max(w_g,1e-6)) + s) = max(wpre,1e-6) * exp(s) / Z
  out = (p^T V)^T -> @ Wo + bo

Orientation on device: scoresT [m(key) partitions, n(query) free].
delta_x/y: pairwise log-dist, 8 freqs x sin/cos via exact fixed-point
range-reduction (2 DVE int ops) + ACT Sin; f->head contraction on PE via
a gather to f-on-partition tiles. delta_w/h are separable -> rank-33
matmuls sharing the same strip PSUM accumulators.
"""
import math

import numpy as np

H = 8
D = 512
DK = 64
NSEQ = 256
BFULL = 16
BL = 2            # batches per core
NCORES = 8
NMT = 2           # m tiles of 128 per batch
PI = math.pi
TWO_PI = 2 * math.pi
SBIG = 128.0      # positivity shift (periods) for fixpoint wrap
CJ = [100.0 * (1000.0 ** (-j / 8.0)) for j in range(8)]

_CACHE = {}


def _build_nc():
    import concourse.bacc as bacc
    import concourse.tile as tile
    from concourse import mybir

    F32 = mybir.dt.float32
    F16 = mybir.dt.float16
    I32 = mybir.dt.int32
    AF = mybir.ActivationFunctionType
    ALU = mybir.AluOpType

    nc = bacc.Bacc("TRN2", target_bir_lowering=False, debug=False)

    # ---- DRAM I/O ----
    xq = nc.dram_tensor("xq", [BL, NSEQ, D], F32, kind="ExternalInput")
    xk = nc.dram_tensor("xk", [BL, NSEQ, D], F32, kind="ExternalInput")
    xv = nc.dram_tensor("xv", [BL, NSEQ, D], F32, kind="ExternalInput")
    nsd = nc.dram_tensor("nsd", [BL, 6, NSEQ], F32, kind="ExternalInput")
    msd = nc.dram_tensor("msd", [BL, 2, NSEQ], F32, kind="ExternalInput")
    mkb = nc.dram_tensor("mkb", [BL, NSEQ], F32, kind="ExternalInput")
    wq = nc.dram_tensor("wq", [D, D], F16, kind="ExternalInput")
    wk = nc.dram_tensor("wk", [D, D], F16, kind="ExternalInput")
    wv = nc.dram_tensor("wv", [D, D], F16, kind="ExternalInput")
    wo = nc.dram_tensor("wo", [D, D], F16, kind="ExternalInput")
    bqc = nc.dram_tensor("bqc", [D], F32, kind="ExternalInput")
    bkc = nc.dram_tensor("bkc", [D], F32, kind="ExternalInput")
    bvc = nc.dram_tensor("bvc", [D], F32, kind="ExternalInput")
    boc = nc.dram_tensor("boc", [D], F32, kind="ExternalInput")
    wpdx = nc.dram_tensor("wpdx", [128, 64], F16, kind="ExternalInput")
    wpdy = nc.dram_tensor("wpdy", [128, 64], F16, kind="ExternalInput")
    mixh = nc.dram_tensor("mixh", [H, 33, 33], F16, kind="ExternalInput")
    fsc = nc.dram_tensor("fsc", [33, 2], F32, kind="ExternalInput")
    iden = nc.dram_tensor("iden", [128, 128], F32, kind="ExternalInput")
    out = nc.dram_tensor("out", [BL, NSEQ, D], F32, kind="ExternalOutput")

    import contextlib
    ctx = contextlib.ExitStack()
    with ctx:
        tc = ctx.enter_context(tile.TileContext(nc))
        singles = ctx.enter_context(tc.tile_pool(name="singles", bufs=1))
        wpool = ctx.enter_context(tc.tile_pool(name="wpool", bufs=1))
        bigseq = ctx.enter_context(tc.tile_pool(name="bigseq", bufs=1))
        work = ctx.enter_context(tc.tile_pool(name="work", bufs=4))
        outp = ctx.enter_context(tc.tile_pool(name="outp", bufs=2))

        # ---------- constants ----------
        npi = singles.tile([128, 1], F32)
        nc.vector.memset(npi[:], -PI)

        ident = singles.tile([128, 128], F32)
        nc.sync.dma_start(out=ident[:], in_=iden[:])

        w_dx = singles.tile([128, 64], F16)
        nc.sync.dma_start(out=w_dx[:], in_=wpdx[:])
        w_dy = singles.tile([128, 64], F16)
        nc.sync.dma_start(out=w_dy[:], in_=wpdy[:])

        mixt = singles.tile([33, H, 33], F16)
        nc.sync.dma_start(out=mixt[:], in_=mixh[:].rearrange("h a b -> a h b"))

        fsc1 = singles.tile([33, 1], F32)
        nc.sync.dma_start(out=fsc1[:], in_=fsc[:, 0:1])
        fsc2 = singles.tile([33, 1], F32)
        nc.sync.dma_start(out=fsc2[:], in_=fsc[:, 1:2])

        import concourse.bass as bass

        def bcast_ap(dram_ap, parts):
            return bass.AP(
                tensor=dram_ap.tensor,
                offset=dram_ap.offset,
                ap=[[0, parts]] + list(dram_ap.ap),
            )

        bo_b = singles.tile([128, D], F32)
        nc.sync.dma_start(out=bo_b[:], in_=bcast_ap(boc[:], 128))
        bv_b = singles.tile([128, D], F32)
        nc.sync.dma_start(out=bv_b[:], in_=bcast_ap(bvc[:], 128))

        bq_cols = singles.tile([128, 4], F32)
        nc.sync.dma_start(out=bq_cols[:], in_=bqc[:].rearrange("(d p) -> p d", p=128))
        bk_cols = singles.tile([128, 4], F32)
        nc.sync.dma_start(out=bk_cols[:], in_=bkc[:].rearrange("(d p) -> p d", p=128))

        # weights to SBUF (fp16), as 4 din-tiles each [128, 512]
        wq_t = [wpool.tile([128, D], F16, tag=f"wq{k}", name=f"wq_t{k}") for k in range(4)]
        wk_t = [wpool.tile([128, D], F16, tag=f"wk{k}", name=f"wk_t{k}") for k in range(4)]
        wv_t = [wpool.tile([128, D], F16, tag=f"wv{k}", name=f"wv_t{k}") for k in range(4)]
        wo_t = [wpool.tile([128, D], F16, tag=f"wo{k}", name=f"wo_t{k}") for k in range(4)]
        for k in range(4):
            nc.sync.dma_start(out=wq_t[k][:], in_=wq[k * 128:(k + 1) * 128, :])
            nc.sync.dma_start(out=wk_t[k][:], in_=wk[k * 128:(k + 1) * 128, :])
            nc.sync.dma_start(out=wv_t[k][:], in_=wv[k * 128:(k + 1) * 128, :])
            nc.sync.dma_start(out=wo_t[k][:], in_=wo[k * 128:(k + 1) * 128, :])

        # n-side broadcast tiles per batch: rows of nsd = cx cy w3 h3 lw lh
        nbc = {}
        for b in range(BL):
            for ri, nm in enumerate(["cx", "cy", "w3", "h3", "lw", "lh"]):
                t = singles.tile([128, NSEQ], F32, tag=f"nbc{b}{nm}")
                nc.sync.dma_start(out=t[:], in_=bcast_ap(nsd[b, ri, :], 128))
                nbc[(b, nm)] = t

        # m-side columns per (b, mt)
        mcol = {}
        for b in range(BL):
            for mt in range(NMT):
                sl = slice(mt * 128, (mt + 1) * 128)
                for ri, nm in enumerate(["cx", "cy"]):
                    t = singles.tile([128, 1], F32, tag=f"mc{b}{mt}{nm}")
                    nc.sync.dma_start(out=t[:], in_=msd[b, ri, sl])
                    mcol[(b, mt, nm)] = t
                t = singles.tile([128, 1], F32, tag=f"mb{b}{mt}")
                nc.sync.dma_start(out=t[:], in_=mkb[b, sl])
                mcol[(b, mt, "mkb")] = t

        # ---------- phase 1: transposes + projections (+ factor/mix) ----------
        ph1ctx = contextlib.ExitStack()
        xtp = ph1ctx.enter_context(tc.tile_pool(name="xtp", bufs=1))
        xt_q = [xtp.tile([128, 512], F16, tag=f"xtq{k}", name=f"xt_q{k}") for k in range(4)]
        xt_k = [xtp.tile([128, 512], F16, tag=f"xtk{k}", name=f"xt_k{k}") for k in range(4)]
        xt_v = [xtp.tile([128, 512], F16, tag=f"xtv{k}", name=f"xt_v{k}") for k in range(4)]
        ph1 = ph1ctx.enter_context(tc.tile_pool(name="ph1ps", bufs=2, space="PSUM"))
        ph1b = ph1ctx.enter_context(tc.tile_pool(name="ph1ps2", bufs=2, space="PSUM"))

        for src, dstl in ((xq, xt_q), (xk, xt_k), (xv, xt_v)):
            flat = src[:].rearrange("b n d -> (b n) d")
            for i in range(4):      # bn tile
                xin = work.tile([128, 512], F32, tag="xin", bufs=3)
                nc.sync.dma_start(out=xin[:], in_=flat[i * 128:(i + 1) * 128, :])
                for k in range(4):  # din tile
                    tp = ph1.tile([128, 128], F32, tag="tp")
                    nc.tensor.transpose(tp[:], xin[:, k * 128:(k + 1) * 128], ident[:])
                    nc.vector.tensor_copy(
                        out=dstl[k][:, i * 128:(i + 1) * 128], in_=tp[:])

        # factor base tiles (B == F) per batch, via fixpoint wrap + sin
        fac = {}
        for b in range(BL):
            lb = work.tile([33, NSEQ], F32, tag="lb", bufs=2)
            nc.sync.dma_start(out=lb[0:16, :], in_=bcast_ap(nsd[b, 4, :], 16))
            nc.sync.dma_start(out=lb[16:33, :], in_=bcast_ap(nsd[b, 5, :], 17))
            ufac = work.tile([33, NSEQ], I32, tag="ufac", bufs=2)
            nc.vector.tensor_scalar(out=ufac[:], in0=lb[:], scalar1=fsc1[:],
                                    scalar2=fsc2[:], op0=ALU.mult, op1=ALU.add)
            ffac = work.tile([33, NSEQ], I32, tag="ffac", bufs=2)
            nc.vector.tensor_scalar(out=ffac[:], in0=ufac[:], scalar1=65535,
                                    scalar2=None, op0=ALU.bitwise_and)
            bf = bigseq.tile([33, NSEQ], F16, tag=f"fac{b}")
            nc.scalar.activation(bf[:], ffac[:], AF.Sin, bias=npi[0:33, :],
                                 scale=TWO_PI / 65536.0)
            nc.vector.memset(bf[32:33, :], 1.0)
            fac[b] = bf

        # mixed m-side factors in strip-column order:
        # mstat[b] [33, 16=(mt,s), 128=(half*64 + h*8 + m')]
        mfac = {}
        for b in range(BL):
            mt_s = bigseq.tile([33, 16, 128], F16, tag=f"mfac{b}",
                               name=f"mfac{b}")
            for h in range(H):
                mp = ph1b.tile([33, NSEQ], F32, tag="mp")
                nc.tensor.matmul(mp[:], mixt[:, h, :], fac[b][:],
                                 start=True, stop=True)
                dst = bass.AP(
                    tensor=mt_s.tensor,
                    offset=mt_s[:].offset + h * 8,
                    ap=[list(mt_s[:].ap[0]),
                        [1024, 2], [128, 8], [64, 2], [1, 8]],
                )
                nc.vector.tensor_copy(out=dst, in_=mp[:])
            mfac[b] = mt_s

        # projections
        qT = [bigseq.tile([128, 512], F16, tag=f"qT{d}", name=f"qT{d}") for d in range(4)]
        kT = [bigseq.tile([128, 512], F16, tag=f"kT{d}", name=f"kT{d}") for d in range(4)]
        for d in range(4):
            pq = ph1b.tile([128, 512], F32, tag="pq")
            for k in range(4):
                nc.tensor.matmul(pq[:], wq_t[k][:, d * 128:(d + 1) * 128],
                                 xt_q[k][:], start=(k == 0), stop=(k == 3))
            nc.vector.tensor_scalar(out=qT[d][:], in0=pq[:],
                                    scalar1=bq_cols[:, d:d + 1], scalar2=None,
                                    op0=ALU.add)
            pk = ph1b.tile([128, 512], F32, tag="pq")
            for k in range(4):
                nc.tensor.matmul(pk[:], wk_t[k][:, d * 128:(d + 1) * 128],
                                 xt_k[k][:], start=(k == 0), stop=(k == 3))
            nc.vector.tensor_scalar(out=kT[d][:], in0=pk[:],
                                    scalar1=bk_cols[:, d:d + 1], scalar2=None,
                                    op0=ALU.add)

        # v projection -> v1 tiles [128, 65] per (b, h, mt), col 64 = ones
        v1 = {}
        for b in range(BL):
            for h in range(H):
                for mt in range(NMT):
                    v1[(b, h, mt)] = bigseq.tile([128, 65], F16,
                                                 tag=f"v1_{b}_{h}_{mt}",
                                                 name=f"v1_{b}_{h}_{mt}")
        for i in range(4):          # bn tile = (b, mt)
            b, mt = divmod(i, 2)
            pv = ph1b.tile([128, 512], F32, tag="pq")
            for k in range(4):
                nc.tensor.matmul(pv[:], xt_v[k][:, i * 128:(i + 1) * 128],
                                 wv_t[k][:], start=(k == 0), stop=(k == 3))
            for h in range(H):
                t = v1[(b, h, mt)]
                nc.vector.scalar_tensor_tensor(
                    out=t[:, 0:64], in0=pv[:, h * 64:(h + 1) * 64], scalar=1.0,
                    in1=bv_b[:, h * 64:(h + 1) * 64], op0=ALU.mult, op1=ALU.add)
                nc.vector.memset(t[:, 64:65], 1.0)

        ph1ctx.close()

        # ---------- phase 2: geo pairwise + attention ----------
        gpool = ctx.enter_context(tc.tile_pool(name="gpool", bufs=2))
        gp2 = ctx.enter_context(tc.tile_pool(name="gp2", bufs=4))
        stp = ctx.enter_context(tc.tile_pool(name="stp", bufs=2))
        wpre_p = ctx.enter_context(tc.tile_pool(name="wpre", bufs=9))
        expp = ctx.enter_context(tc.tile_pool(name="expp", bufs=3))
        punp = ctx.enter_context(tc.tile_pool(name="punp", bufs=17))
        ph2a = ctx.enter_context(tc.tile_pool(name="scps", bufs=2, space="PSUM"))
        ph2b = ctx.enter_context(tc.tile_pool(name="stps", bufs=2, space="PSUM"))
        ph2c = ctx.enter_context(tc.tile_pool(name="pvps", bufs=2, space="PSUM"))
        ph2d = ctx.enter_context(tc.tile_pool(name="pops", bufs=2, space="PSUM"))

        attn_oT = {b: [bigseq.tile([128, NSEQ], F16, tag=f"aoT{b}{k}",
                                   name=f"aoT{b}_{k}")
                       for k in range(4)] for b in range(BL)}

        for b in range(BL):
            pun = {}
            for mt in range(NMT):
                # --- deltas ---
                dx = work.tile([128, 512], F32, tag="dx", bufs=3)  # [:,0:256]=dx, dy
                t1 = work.tile([128, 256], F32, tag="t1", bufs=2)
                for ci, (cn, w3n, lwn) in enumerate(
                        (("cx", "w3", "lw"), ("cy", "h3", "lh"))):
                    cm = mcol[(b, mt, cn)]
                    nc.vector.tensor_scalar(
                        out=t1[:], in0=nbc[(b, cn)][:], scalar1=cm[:],
                        scalar2=None, op0=ALU.subtract)
                    t2 = work.tile([128, 256], F32, tag="t2", bufs=2)
                    nc.vector.scalar_tensor_tensor(
                        out=t2[:], in0=t1[:], scalar=-1.0, in1=t1[:],
                        op0=ALU.mult, op1=ALU.max)
                    t3 = work.tile([128, 256], F32, tag="t3", bufs=2)
                    nc.vector.tensor_tensor(
                        out=t3[:], in0=t2[:], in1=nbc[(b, w3n)][:], op=ALU.max)
                    t4 = work.tile([128, 256], F32, tag="t4", bufs=2)
                    nc.scalar.activation(t4[:], t3[:], AF.Ln)
                    nc.vector.tensor_tensor(
                        out=dx[:, ci * 256:(ci + 1) * 256], in0=t4[:],
                        in1=nbc[(b, lwn)][:], op=ALU.subtract)

                # --- wrap + sin -> G [128, 16, 512] fp16 ---
                gt = gpool.tile([128, 16, 512], F16, tag="gt")
                for j in range(8):
                    for trig in range(2):
                        f = j * 2 + trig
                        s1 = CJ[j] * 65536.0 / TWO_PI
                        s2 = (trig * 0.25 + 0.5 + SBIG) * 65536.0
                        uu = work.tile([128, 512], I32, tag="uu", bufs=3)
                        nc.vector.tensor_scalar(
                            out=uu[:], in0=dx[:], scalar1=s1, scalar2=s2,
                            op0=ALU.mult, op1=ALU.add)
                        ff = work.tile([128, 512], I32, tag="ff", bufs=3)
                        nc.vector.tensor_scalar(
                            out=ff[:], in0=uu[:], scalar1=65535, scalar2=None,
                            op0=ALU.bitwise_and)
                        nc.scalar.activation(gt[:, f, :], ff[:], AF.Sin,
                                             bias=npi[:], scale=TWO_PI / 65536.0)

                # --- gather to f-on-partitions + contraction -> strips ---
                strip_s = stp.tile([128, 8, NSEQ], F32, tag="strip_s")
                for s in range(8):
                    sp = ph2b.tile([128, NSEQ], F32, tag="sp")
                    for half in range(2):
                        t = s * 2 + half
                        gp = gp2.tile([128, 512], F16, tag="gp")
                        nc.sync.dma_start(
                            out=gp[:], in_=gt[t * 8:(t + 1) * 8, :, :])
                        nc.tensor.matmul(sp[half * 64:(half + 1) * 64, :],
                                         w_dx[:], gp[:, 0:256],
                                         start=True, stop=False)
                        nc.tensor.matmul(sp[half * 64:(half + 1) * 64, :],
                                         w_dy[:], gp[:, 256:512],
                                         start=False, stop=False)
                    # separable comps + bias, full 128 rows
                    nc.tensor.matmul(
                        sp[:], mfac[b][:, mt * 8 + s, :],
                        fac[b][:], start=False, stop=True)
                    nc.vector.tensor_copy(out=strip_s[:, s, :], in_=sp[:])

                # --- scatter strips -> wpre per h ---
                wpre = {}
                for h in range(H):
                    wp = wpre_p.tile([128, NSEQ], F32, tag="wp")
                    for s in range(8):
                        for half in range(2):
                            nc.sync.dma_start(
                                out=wp[s * 16 + half * 8:s * 16 + half * 8 + 8, :],
                                in_=strip_s[half * 64 + h * 8:
                                            half * 64 + h * 8 + 8, s, :])
                    wpre[h] = wp

                # --- QK + exp + p_un ---
                for h in range(H):
                    d, r = divmod(h, 2)
                    hs = slice(r * 64, (r + 1) * 64)
                    ns = slice(b * 256, (b + 1) * 256)
                    sc = ph2a.tile([128, NSEQ], F32, tag="sc")
                    mb = b * 256 + mt * 128
                    nc.tensor.matmul(
                        sc[:], kT[d][hs, mb:mb + 128],
                        qT[d][hs, ns], start=True, stop=True)
                    ex = expp.tile([128, NSEQ], F32, tag="ex")
                    nc.scalar.activation(ex[:], sc[:], AF.Exp,
                                         bias=mcol[(b, mt, "mkb")][:])
                    pu = punp.tile([128, NSEQ], F16, tag="pu")
                    nc.vector.scalar_tensor_tensor(
                        out=pu[:], in0=wpre[h][:], scalar=1e-6, in1=ex[:],
                        op0=ALU.max, op1=ALU.mult)
                    pun[(h, mt)] = pu

            # --- PV + normalize ---
            for h in range(H):
                pv = ph2c.tile([65, NSEQ], F32, tag="pvp")
                for mt in range(NMT):
                    nc.tensor.matmul(pv[:], v1[(b, h, mt)][:], pun[(h, mt)][:],
                                     start=(mt == 0), stop=(mt == 1))
                rc = work.tile([1, NSEQ], F32, tag="rc", bufs=2)
                nc.vector.reciprocal(out=rc[:], in_=pv[64:65, :])
                rcb = work.tile([64, NSEQ], F32, tag="rcb", bufs=2)
                nc.sync.dma_start(
                    out=rcb[:],
                    in_=bass.AP(tensor=rc.tensor, offset=rc[:].offset,
                                ap=[list(rc[:].ap[0]), [0, 64]]
                                + list(rc[:].ap[1:])))
                d, r = divmod(h, 2)
                nc.vector.tensor_tensor(
                    out=attn_oT[b][d][r * 64:(r + 1) * 64, :],
                    in0=pv[0:64, :], in1=rcb[:], op=ALU.mult)

        # ---------- phase 3: output projection ----------
        for b in range(BL):
            for bnt in range(NMT):
                po = ph2d.tile([128, D], F32, tag="po")
                for k in range(4):
                    nc.tensor.matmul(
                        po[:], attn_oT[b][k][:, bnt * 128:(bnt + 1) * 128],
                        wo_t[k][:], start=(k == 0), stop=(k == 3))
                ot = outp.tile([128, D], F32, tag="ot")
                nc.vector.tensor_tensor(out=ot[:], in0=po[:], in1=bo_b[:],
                                        op=ALU.add)
                nc.sync.dma_start(
                    out=out[b, bnt * 128:(bnt + 1) * 128, :], in_=ot[:])

    return nc


def _host_prep(inputs):
    """Build the per-core input maps."""
    iq = np.ascontiguousarray(inputs["input_query"], dtype=np.float32)
    ik = np.ascontiguousarray(inputs["input_key"], dtype=np.float32)
    iv = np.ascontiguousarray(inputs["input_value"], dtype=np.float32)
    box = np.asarray(inputs["input_box"], dtype=np.float32)
    mask = np.asarray(inputs["mask"])
    Wq = np.asarray(inputs["Wq"], dtype=np.float32)
    bq = np.asarray(inputs["bq"], dtype=np.float32)
    Wk = np.asarray(inputs["Wk"], dtype=np.float32)
    bk = np.asarray(inputs["bk"], dtype=np.float32)
    Wv = np.asarray(inputs["Wv"], dtype=np.float32)
    bv = np.asarray(inputs["bv"], dtype=np.float32)
    Wo = np.asarray(inputs["Wo"], dtype=np.float32)
    bo = np.asarray(inputs["bo"], dtype=np.float32)
    WG_w = np.asarray(inputs["WG_w"], dtype=np.float32)
    WG_b = np.asarray(inputs["WG_b"], dtype=np.float32)

    scale = 1.0 / math.sqrt(DK)
    wq16 = (Wq * scale).astype(np.float16)
    wk16 = Wk.astype(np.float16)
    wv16 = Wv.astype(np.float16)
    wo16 = Wo.astype(np.float16)
    bq_s = (bq * scale).astype(np.float32)

    # box-derived per-box quantities
    x_min, y_min, x_max, y_max = [box[..., i] for i in range(4)]
    cx = (x_min + x_max) * 0.5
    cy = (y_min + y_max) * 0.5
    w = x_max - x_min + 1.0
    hh = y_max - y_min + 1.0
    nsd = np.stack([cx, cy, 1e-3 * w, 1e-3 * hh,
                    np.log(w), np.log(hh)], axis=1).astype(np.float32)
    msd = np.stack([cx, cy], axis=1).astype(np.float32)
    mkb = ((mask.astype(np.float32) - 1.0) * 1e9).astype(np.float32)

    # pairwise contraction weights W'_dx/dy [128=(m'16 x f16... m'*16+f), 64=(h,m')]
    wpdx = np.zeros((128, 64), dtype=np.float32)
    wpdy = np.zeros((128, 64), dtype=np.float32)
    for mp in range(8):
        for j in range(8):
            for trig in range(2):
                f = j * 2 + trig
                row = mp * 16 + f
                for h in range(H):
                    col = h * 8 + mp
                    wpdx[row, col] = WG_w[h, (32 * trig) + 0 * 8 + j]
                    wpdy[row, col] = WG_w[h, (32 * trig) + 1 * 8 + j]
    wpdx = wpdx.astype(np.float16)
    wpdy = wpdy.astype(np.float16)

    # mix matrices for separable comps: [H, 33(rb), 33(rc)]
    mixh = np.zeros((H, 33, 33), dtype=np.float32)
    for h in range(H):
        for i in (2, 3):
            for j in range(8):
                rb_s = (i - 2) * 16 + j * 2 + 0
                rb_c = rb_s + 1
                ws = WG_w[h, i * 8 + j]
                wc = WG_w[h, 32 + i * 8 + j]
                rc0 = (i - 2) * 16 + j * 2 + 0
                rc1 = rc0 + 1
                mixh[h, rb_c, rc0] = ws
                mixh[h, rb_s, rc0] = wc
                mixh[h, rb_c, rc1] = wc
                mixh[h, rb_s, rc1] = -ws
        mixh[h, 32, 32] = WG_b[h]
    mixh = mixh.astype(np.float16)

    # factor-build fixpoint constants per row r=(i,j,trig), i in {2,3}
    fsc = np.zeros((33, 2), dtype=np.float32)
    for i in (2, 3):
        for j in range(8):
            for trig in range(2):
                r = (i - 2) * 16 + j * 2 + trig
                fsc[r, 0] = CJ[j] * 65536.0 / TWO_PI
                fsc[r, 1] = (trig * 0.25 + 0.5 + SBIG) * 65536.0
    fsc[32, 0] = 0.0
    fsc[32, 1] = (0.5 + SBIG) * 65536.0

    iden = np.eye(128, dtype=np.float32)

    shared = dict(wq=wq16, wk=wk16, wv=wv16, wo=wo16, bqc=bq_s, bkc=bk,
                  bvc=bv, boc=bo, wpdx=wpdx, wpdy=wpdy, mixh=mixh, fsc=fsc,
                  iden=iden)
    in_maps = []
    for c in range(NCORES):
        sl = slice(c * BL, (c + 1) * BL)
        m = dict(shared)
        m.update(xq=iq[sl], xk=ik[sl], xv=iv[sl], nsd=nsd[sl], msd=msd[sl],
                 mkb=mkb[sl])
        in_maps.append(m)
    return in_maps


def kernel(**inputs):
    from concourse.bass_utils import run_bass_kernel_spmd

    if "nc" not in _CACHE:
        nc = _build_nc()
        nc.finalize()
        _CACHE["nc"] = nc
    nc = _CACHE["nc"]

    in_maps = _host_prep(inputs)
    res = run_bass_kernel_spmd(nc, in_maps, list(range(NCORES)))
    outs = [res.results[c]["out"] for c in range(NCORES)]
    return np.concatenate(outs, axis=0).astype(np.float32)


if __name__ == "__main__":
    rng = np.random.default_rng(0)
    # smoke build only
    nc = _build_nc()
    nc.finalize()
    print("build ok")


# revision 23
# speedup vs baseline: 1.9223x; 1.9223x over previous
"""BoxMultiHeadedAttention Trainium2 kernel.

Self-contained: takes FULL inputs, shards batch 16 -> 8 cores x 2,
runs a Bass/Tile kernel per core via run_bass_kernel_spmd, gathers.

Orientation on device: scoresT [m(key) partitions, n(query) free].
Key rows use a permuted order r = m'*8 + s (box = s*16 + m') so that
the strip->wpre reassembly is a single DMA per (b, mt, head).
delta_x/y: pairwise log-dist, 8 freqs x sin/cos via exact fixed-point
range reduction (2 DVE int ops) + ACT Sin; f->head contraction on PE
via a gathered f-on-partition layout. delta_w/h are separable ->
rank-33 matmuls accumulating into the same strip PSUM.
"""
import math

import numpy as np

H = 8
D = 512
DK = 64
NSEQ = 256
BL = 2            # batches per core
NCORES = 8
NMT = 2           # m tiles of 128 per batch
PI = math.pi
TWO_PI = 2 * math.pi
SBIG = 128.0      # positivity shift (periods) for fixpoint wrap
CJ = [100.0 * (1000.0 ** (-j / 8.0)) for j in range(8)]

# key-row permutation within a 128-row m tile: row r <-> box psi(r)
PSI = [(r % 8) * 16 + r // 8 for r in range(128)]   # box index for row r

_CACHE = {}


def _build_nc():
    import contextlib

    import concourse.bacc as bacc
    import concourse.bass as bass
    import concourse.tile as tile
    from concourse import mybir

    F32 = mybir.dt.float32
    F16 = mybir.dt.float16
    I32 = mybir.dt.int32
    AF = mybir.ActivationFunctionType
    ALU = mybir.AluOpType

    nc = bacc.Bacc("TRN2", target_bir_lowering=False, debug=False)

    xq = nc.dram_tensor("xq", [BL, NSEQ, D], F32, kind="ExternalInput")
    xk = nc.dram_tensor("xk", [BL, NSEQ, D], F32, kind="ExternalInput")
    xv = nc.dram_tensor("xv", [BL, NSEQ, D], F32, kind="ExternalInput")
    nsd = nc.dram_tensor("nsd", [BL, 6, NSEQ], F32, kind="ExternalInput")
    msd = nc.dram_tensor("msd", [BL, 2, NSEQ], F32, kind="ExternalInput")
    mkb = nc.dram_tensor("mkb", [BL, NSEQ], F32, kind="ExternalInput")
    wq = nc.dram_tensor("wq", [D, D], F16, kind="ExternalInput")
    wk = nc.dram_tensor("wk", [D, D], F16, kind="ExternalInput")
    wv = nc.dram_tensor("wv", [D, D], F16, kind="ExternalInput")
    wo = nc.dram_tensor("wo", [D, D], F16, kind="ExternalInput")
    bqc = nc.dram_tensor("bqc", [D], F32, kind="ExternalInput")
    bkc = nc.dram_tensor("bkc", [D], F32, kind="ExternalInput")
    bvc = nc.dram_tensor("bvc", [D], F32, kind="ExternalInput")
    boc = nc.dram_tensor("boc", [D], F32, kind="ExternalInput")
    # 4 contraction weight mats [128,128]: (comp dx/dy) x (f-half)
    wpc = nc.dram_tensor("wpc", [4, 128, 128], F16, kind="ExternalInput")
    mixh = nc.dram_tensor("mixh", [H, 33, 33], F16, kind="ExternalInput")
    fsc = nc.dram_tensor("fsc", [33, 2], F32, kind="ExternalInput")
    iden = nc.dram_tensor("iden", [128, 128], F32, kind="ExternalInput")
    out = nc.dram_tensor("out", [BL, NSEQ, D], F32, kind="ExternalOutput")

    ctx = contextlib.ExitStack()
    with ctx:
        tc = ctx.enter_context(tile.TileContext(nc))
        singles = ctx.enter_context(tc.tile_pool(name="singles", bufs=1))
        wpool = ctx.enter_context(tc.tile_pool(name="wpool", bufs=1))
        bigseq = ctx.enter_context(tc.tile_pool(name="bigseq", bufs=1))
        work = ctx.enter_context(tc.tile_pool(name="work", bufs=4))
        outp = ctx.enter_context(tc.tile_pool(name="outp", bufs=2))

        # ---------- constants ----------
        npi = singles.tile([128, 1], F32)
        nc.vector.memset(npi[:], -PI)
        ident = singles.tile([128, 128], F32)
        nc.sync.dma_start(out=ident[:], in_=iden[:])
        wpct = singles.tile([128, 4, 128], F16)
        nc.sync.dma_start(out=wpct[:], in_=wpc[:].rearrange("c r m -> r c m"))
        mixt = singles.tile([33, H, 33], F16)
        nc.sync.dma_start(out=mixt[:], in_=mixh[:].rearrange("h a b -> a h b"))
        fsc1 = singles.tile([33, 1], F32)
        nc.sync.dma_start(out=fsc1[:], in_=fsc[:, 0:1])
        fsc2 = singles.tile([33, 1], F32)
        nc.sync.dma_start(out=fsc2[:], in_=fsc[:, 1:2])

        def bcast_ap(dram_ap, parts):
            return bass.AP(
                tensor=dram_ap.tensor,
                offset=dram_ap.offset,
                ap=[[0, parts]] + list(dram_ap.ap),
            )

        bo_b = singles.tile([128, D], F32)
        nc.sync.dma_start(out=bo_b[:], in_=bcast_ap(boc[:], 128))
        bv_b = singles.tile([128, D], F32)
        nc.sync.dma_start(out=bv_b[:], in_=bcast_ap(bvc[:], 128))
        bq_cols = singles.tile([128, 4], F32)
        nc.sync.dma_start(out=bq_cols[:], in_=bqc[:].rearrange("(d p) -> p d", p=128))
        bk_cols = singles.tile([128, 4], F32)
        nc.sync.dma_start(out=bk_cols[:], in_=bkc[:].rearrange("(d p) -> p d", p=128))

        wo_t = [wpool.tile([128, D], F16, tag=f"wo{k}", name=f"wo_t{k}") for k in range(4)]
        for k in range(4):
            nc.sync.dma_start(out=wo_t[k][:], in_=wo[k * 128:(k + 1) * 128, :])

        # n-side broadcast tiles per batch (natural box order)
        nbc = {}
        for b in range(BL):
            for ri, nm in enumerate(["cx", "cy", "w3", "h3", "lw", "lh"]):
                t = singles.tile([128, NSEQ], F32, tag=f"nbc{b}{nm}")
                nc.sync.dma_start(out=t[:], in_=bcast_ap(nsd[b, ri, :], 128))
                nbc[(b, nm)] = t

        # m-side columns per (b, mt) -- host supplies PSI-permuted values
        mcol = {}
        for b in range(BL):
            for mt in range(NMT):
                sl = slice(mt * 128, (mt + 1) * 128)
                for ri, nm in enumerate(["cx", "cy"]):
                    t = singles.tile([128, 1], F32, tag=f"mc{b}{mt}{nm}")
                    nc.sync.dma_start(out=t[:], in_=msd[b, ri, sl])
                    mcol[(b, mt, nm)] = t
                t = singles.tile([128, 1], F32, tag=f"mb{b}{mt}")
                nc.sync.dma_start(out=t[:], in_=mkb[b, sl])
                mcol[(b, mt, "mkb")] = t

        # ---------- phase 1 ----------
        ph1ctx = contextlib.ExitStack()
        xtp = ph1ctx.enter_context(tc.tile_pool(name="xtp", bufs=1))
        xt_q = [xtp.tile([128, 512], F16, tag=f"xtq{k}", name=f"xt_q{k}") for k in range(4)]
        xt_k = [xtp.tile([128, 512], F16, tag=f"xtk{k}", name=f"xt_k{k}") for k in range(4)]
        xt_v = [xtp.tile([128, 512], F16, tag=f"xtv{k}", name=f"xt_v{k}") for k in range(4)]
        ph1 = ph1ctx.enter_context(tc.tile_pool(name="ph1ps", bufs=2, space="PSUM"))
        ph1b = ph1ctx.enter_context(tc.tile_pool(name="ph1ps2", bufs=2, space="PSUM"))
        w1pool = ph1ctx.enter_context(tc.tile_pool(name="w1pool", bufs=1))
        work1 = ph1ctx.enter_context(tc.tile_pool(name="work1", bufs=3))
        wq_t = [w1pool.tile([128, D], F16, tag=f"wq{k}", name=f"wq_t{k}") for k in range(4)]
        wk_t = [w1pool.tile([128, D], F16, tag=f"wk{k}", name=f"wk_t{k}") for k in range(4)]
        wv_t = [w1pool.tile([128, D], F16, tag=f"wv{k}", name=f"wv_t{k}") for k in range(4)]
        for k in range(4):
            nc.sync.dma_start(out=wq_t[k][:], in_=wq[k * 128:(k + 1) * 128, :])
            nc.sync.dma_start(out=wk_t[k][:], in_=wk[k * 128:(k + 1) * 128, :])
            nc.sync.dma_start(out=wv_t[k][:], in_=wv[k * 128:(k + 1) * 128, :])

        for src, dstl, perm in ((xq, xt_q, False), (xk, xt_k, True),
                                (xv, xt_v, True)):
            flat = src[:].rearrange("b n d -> (b n) d")
            for i in range(4):      # bn tile
                xin = work1.tile([128, 512], F32, tag="xin", bufs=3)
                nc.sync.dma_start(out=xin[:], in_=flat[i * 128:(i + 1) * 128, :])
                for k in range(4):  # din tile
                    tp = ph1.tile([128, 128], F32, tag="tp")
                    nc.tensor.transpose(tp[:], xin[:, k * 128:(k + 1) * 128], ident[:])
                    dst = dstl[k]
                    if perm:
                        # write cols permuted: input col j=(s,m') -> r=m'*8+s
                        dap = bass.AP(
                            tensor=dst.tensor,
                            offset=dst[:].offset + i * 128,
                            ap=[list(dst[:].ap[0]), [1, 8], [8, 16]],
                        )
                        nc.vector.tensor_copy(out=dap, in_=tp[:])
                    else:
                        nc.vector.tensor_copy(
                            out=dst[:, i * 128:(i + 1) * 128], in_=tp[:])

        # factor base tiles (shared n/m side) per batch
        fac = {}
        for b in range(BL):
            lb = work1.tile([33, NSEQ], F32, tag="lb", bufs=2)
            nc.sync.dma_start(out=lb[0:16, :], in_=bcast_ap(nsd[b, 4, :], 16))
            nc.sync.dma_start(out=lb[16:33, :], in_=bcast_ap(nsd[b, 5, :], 17))
            ufac = work1.tile([33, NSEQ], I32, tag="ufac", bufs=2)
            nc.vector.tensor_scalar(out=ufac[:], in0=lb[:], scalar1=fsc1[:],
                                    scalar2=fsc2[:], op0=ALU.mult, op1=ALU.add)
            ffac = work1.tile([33, NSEQ], I32, tag="ffac", bufs=2)
            nc.vector.tensor_scalar(out=ffac[:], in0=ufac[:], scalar1=65535,
                                    scalar2=None, op0=ALU.bitwise_and)
            bf = bigseq.tile([33, NSEQ], F16, tag=f"fac{b}", name=f"fac{b}")
            nc.scalar.activation(bf[:], ffac[:], AF.Sin, bias=npi[0:33, :],
                                 scale=TWO_PI / 65536.0)
            nc.vector.memset(bf[32:33, :], 1.0)
            fac[b] = bf

        # mixed m-side factors, strip-column order:
        # mstat[b] [33, 16=(mt,s), 128=(h*16 + m')]
        mfac = {}
        for b in range(BL):
            mt_s = bigseq.tile([33, 16, 128], F16, tag=f"mfac{b}",
                               name=f"mfac{b}")
            for h in range(H):
                mp = ph1b.tile([33, NSEQ], F32, tag="mp")
                nc.tensor.matmul(mp[:], mixt[:, h, :], fac[b][:],
                                 start=True, stop=True)
                dst = bass.AP(
                    tensor=mt_s.tensor,
                    offset=mt_s[:].offset + h * 16,
                    ap=[list(mt_s[:].ap[0]),
                        [1024, 2], [128, 8], [1, 16]],
                )
                nc.vector.tensor_copy(out=dst, in_=mp[:])
            mfac[b] = mt_s

        # projections
        qT = [bigseq.tile([128, 512], F16, tag=f"qT{d}", name=f"qT{d}") for d in range(4)]
        kT = [bigseq.tile([128, 512], F16, tag=f"kT{d}", name=f"kT{d}") for d in range(4)]
        for d in range(4):
            pq = ph1b.tile([128, 512], F32, tag="pq")
            for k in range(4):
                nc.tensor.matmul(pq[:], wq_t[k][:, d * 128:(d + 1) * 128],
                                 xt_q[k][:], start=(k == 0), stop=(k == 3))
            nc.vector.tensor_scalar(out=qT[d][:], in0=pq[:],
                                    scalar1=bq_cols[:, d:d + 1], scalar2=None,
                                    op0=ALU.add)
            pk = ph1b.tile([128, 512], F32, tag="pq")
            for k in range(4):
                nc.tensor.matmul(pk[:], wk_t[k][:, d * 128:(d + 1) * 128],
                                 xt_k[k][:], start=(k == 0), stop=(k == 3))
            nc.vector.tensor_scalar(out=kT[d][:], in0=pk[:],
                                    scalar1=bk_cols[:, d:d + 1], scalar2=None,
                                    op0=ALU.add)

        v1 = {}
        for b in range(BL):
            for h in range(H):
                for mt in range(NMT):
                    v1[(b, h, mt)] = bigseq.tile(
                        [128, 65], F16, tag=f"v1_{b}_{h}_{mt}",
                        name=f"v1_{b}_{h}_{mt}")
        for i in range(4):          # bn tile = (b, mt), rows PSI-permuted
            b, mt = divmod(i, 2)
            pv = ph1b.tile([128, 512], F32, tag="pq")
            for k in range(4):
                nc.tensor.matmul(pv[:], xt_v[k][:, i * 128:(i + 1) * 128],
                                 wv_t[k][:], start=(k == 0), stop=(k == 3))
            for h in range(H):
                t = v1[(b, h, mt)]
                nc.vector.scalar_tensor_tensor(
                    out=t[:, 0:64], in0=pv[:, h * 64:(h + 1) * 64], scalar=1.0,
                    in1=bv_b[:, h * 64:(h + 1) * 64], op0=ALU.mult, op1=ALU.add)
                nc.vector.memset(t[:, 64:65], 1.0)

        ph1ctx.close()

        # ---------- phase 2 ----------
        gpool = ctx.enter_context(tc.tile_pool(name="gpool", bufs=2))
        gp2 = ctx.enter_context(tc.tile_pool(name="gp2", bufs=4))
        stp = ctx.enter_context(tc.tile_pool(name="stp", bufs=2))
        wpre_p = ctx.enter_context(tc.tile_pool(name="wpre", bufs=16))
        expp = ctx.enter_context(tc.tile_pool(name="expp", bufs=3))
        punp = ctx.enter_context(tc.tile_pool(name="punp", bufs=16))
        ph2a = ctx.enter_context(tc.tile_pool(name="scps", bufs=2, space="PSUM"))
        ph2b = ctx.enter_context(tc.tile_pool(name="stps", bufs=2, space="PSUM"))
        ph2c = ctx.enter_context(tc.tile_pool(name="pvps", bufs=1, space="PSUM"))
        ph2d = ctx.enter_context(tc.tile_pool(name="pops", bufs=1, space="PSUM"))

        attn_oT = {b: [bigseq.tile([128, NSEQ], F16, tag=f"aoT{b}{k}",
                                   name=f"aoT{b}_{k}")
                       for k in range(4)] for b in range(BL)}

        for b in range(BL):
            # --- deltas (Ln) for both mt first (table-set batching) ---
            dxs = {}
            for mt in range(NMT):
                dx = work.tile([128, 512], F32, tag="dx", bufs=3)
                t1 = work.tile([128, 256], F32, tag="t1", bufs=2)
                for ci, (cn, w3n, lwn) in enumerate(
                        (("cx", "w3", "lw"), ("cy", "h3", "lh"))):
                    cm = mcol[(b, mt, cn)]
                    nc.vector.tensor_scalar(
                        out=t1[:], in0=nbc[(b, cn)][:], scalar1=cm[:],
                        scalar2=None, op0=ALU.subtract)
                    t2 = work.tile([128, 256], F32, tag="t2", bufs=2)
                    nc.vector.scalar_tensor_tensor(
                        out=t2[:], in0=t1[:], scalar=-1.0, in1=t1[:],
                        op0=ALU.mult, op1=ALU.max)
                    t3 = work.tile([128, 256], F32, tag="t3", bufs=2)
                    nc.vector.tensor_tensor(
                        out=t3[:], in0=t2[:], in1=nbc[(b, w3n)][:], op=ALU.max)
                    t4 = work.tile([128, 256], F32, tag="t4", bufs=2)
                    nc.scalar.activation(t4[:], t3[:], AF.Ln)
                    nc.vector.tensor_tensor(
                        out=dx[:, ci * 256:(ci + 1) * 256], in0=t4[:],
                        in1=nbc[(b, lwn)][:], op=ALU.subtract)
                dxs[mt] = dx

            # --- wrap + sin (Sin set) ---
            gts = {}
            for mt in range(NMT):
                gt = gpool.tile([128, 16, 512], F16, tag="gt")
                for j in range(8):
                    for trig in range(2):
                        f = j * 2 + trig
                        s1 = CJ[j] * 65536.0 / TWO_PI
                        s2 = (trig * 0.25 + 0.5 + SBIG) * 65536.0
                        uu = work.tile([128, 512], I32, tag="uu", bufs=3)
                        nc.vector.tensor_scalar(
                            out=uu[:], in0=dxs[mt][:], scalar1=s1, scalar2=s2,
                            op0=ALU.mult, op1=ALU.add)
                        ff = work.tile([128, 512], I32, tag="ff", bufs=3)
                        nc.vector.tensor_scalar(
                            out=ff[:], in0=uu[:], scalar1=65535, scalar2=None,
                            op0=ALU.bitwise_and)
                        nc.scalar.activation(gt[:, f, :], ff[:], AF.Sin,
                                             bias=npi[:], scale=TWO_PI / 65536.0)
                gts[mt] = gt

            # --- gather + contraction + strips + wpre (no ACT) ---
            wpre = {}
            for mt in range(NMT):
                strip_s = stp.tile([128, 8, NSEQ], F32, tag="strip_s")
                for s in range(8):
                    sp = ph2b.tile([128, NSEQ], F32, tag="sp")
                    first = True
                    for fh in range(2):
                        gp = gp2.tile([128, 512], F16, tag="gp")
                        nc.sync.dma_start(
                            out=gp[:],
                            in_=gts[mt][s::8, fh * 8:(fh + 1) * 8, :])
                        for comp in range(2):
                            nc.tensor.matmul(
                                sp[:], wpct[:, comp * 2 + fh, :],
                                gp[:, comp * 256:(comp + 1) * 256],
                                start=first, stop=False)
                            first = False
                    nc.tensor.matmul(
                        sp[:], mfac[b][:, mt * 8 + s, :],
                        fac[b][:], start=False, stop=True)
                    nc.vector.tensor_copy(out=strip_s[:, s, :], in_=sp[:])
                for h in range(H):
                    wp = wpre_p.tile([128, NSEQ], F32, tag="wp")
                    nc.sync.dma_start(
                        out=wp[:], in_=strip_s[h * 16:(h + 1) * 16, :, :])
                    wpre[(h, mt)] = wp

            # --- QK + exp + p_un (Exp set) ---
            pun = {}
            for mt in range(NMT):
                for h in range(H):
                    d, r = divmod(h, 2)
                    hs = slice(r * 64, (r + 1) * 64)
                    ns = slice(b * 256, (b + 1) * 256)
                    mb = b * 256 + mt * 128
                    sc = ph2a.tile([128, NSEQ], F32, tag="sc")
                    nc.tensor.matmul(sc[:], kT[d][hs, mb:mb + 128],
                                     qT[d][hs, ns], start=True, stop=True)
                    ex = expp.tile([128, NSEQ], F32, tag="ex")
                    nc.scalar.activation(ex[:], sc[:], AF.Exp,
                                         bias=mcol[(b, mt, "mkb")][:])
                    pu = punp.tile([128, NSEQ], F16, tag="pu")
                    nc.vector.scalar_tensor_tensor(
                        out=pu[:], in0=wpre[(h, mt)][:], scalar=1e-6, in1=ex[:],
                        op0=ALU.max, op1=ALU.mult)
                    pun[(h, mt)] = pu

            # --- PV + normalize (in-place on attn_oT) ---
            rcs = work.tile([1, 8, NSEQ], F32, tag="rcs", bufs=2)
            for h in range(H):
                pv = ph2c.tile([65, NSEQ], F32, tag="pvp")
                for mt in range(NMT):
                    nc.tensor.matmul(pv[:], v1[(b, h, mt)][:], pun[(h, mt)][:],
                                     start=(mt == 0), stop=(mt == 1))
                nc.vector.tensor_copy(out=rcs[0:1, h, :], in_=pv[64:65, :])
                d, r = divmod(h, 2)
                nc.vector.tensor_copy(
                    out=attn_oT[b][d][r * 64:(r + 1) * 64, :], in_=pv[0:64, :])
            # 1/z via exp(-ln z) on ACT (Ln/Exp share a table set)
            rln = work.tile([1, 8, NSEQ], F32, tag="rln", bufs=2)
            nc.scalar.activation(rln[:], rcs[:], AF.Ln)
            rrec = work.tile([1, 8, NSEQ], F32, tag="rrec", bufs=2)
            nc.scalar.activation(rrec[:], rln[:], AF.Exp, scale=-1.0)
            for h in range(H):
                rcb = work.tile([128, NSEQ], F32, tag="rcb", bufs=2)
                nc.sync.dma_start(
                    out=rcb[:],
                    in_=bass.AP(tensor=rrec.tensor,
                                offset=rrec[0:1, h, :].offset,
                                ap=[list(rrec[0:1, h, :].ap[0]),
                                    [0, 128], [1, NSEQ]]))
                d, r = divmod(h, 2)
                sl = attn_oT[b][d][r * 64:(r + 1) * 64, :]
                nc.vector.tensor_tensor(
                    out=sl, in0=sl, in1=rcb[r * 64:(r + 1) * 64, :],
                    op=ALU.mult)

        # ---------- phase 3: output projection ----------
        for b in range(BL):
            for bnt in range(NMT):
                po = ph2d.tile([128, D], F32, tag="po")
                for k in range(4):
                    nc.tensor.matmul(
                        po[:], attn_oT[b][k][:, bnt * 128:(bnt + 1) * 128],
                        wo_t[k][:], start=(k == 0), stop=(k == 3))
                ot = outp.tile([128, D], F32, tag="ot")
                nc.vector.tensor_tensor(out=ot[:], in0=po[:], in1=bo_b[:],
                                        op=ALU.add)
                nc.sync.dma_start(
                    out=out[b, bnt * 128:(bnt + 1) * 128, :], in_=ot[:])

    return nc


def _host_prep(inputs):
    iq = np.ascontiguousarray(inputs["input_query"], dtype=np.float32)
    ik = np.ascontiguousarray(inputs["input_key"], dtype=np.float32)
    iv = np.ascontiguousarray(inputs["input_value"], dtype=np.float32)
    box = np.asarray(inputs["input_box"], dtype=np.float32)
    mask = np.asarray(inputs["mask"])
    Wq = np.asarray(inputs["Wq"], dtype=np.float32)
    bq = np.asarray(inputs["bq"], dtype=np.float32)
    Wk = np.asarray(inputs["Wk"], dtype=np.float32)
    bk = np.asarray(inputs["bk"], dtype=np.float32)
    Wv = np.asarray(inputs["Wv"], dtype=np.float32)
    bv = np.asarray(inputs["bv"], dtype=np.float32)
    Wo = np.asarray(inputs["Wo"], dtype=np.float32)
    bo = np.asarray(inputs["bo"], dtype=np.float32)
    WG_w = np.asarray(inputs["WG_w"], dtype=np.float32)
    WG_b = np.asarray(inputs["WG_b"], dtype=np.float32)

    scale = 1.0 / math.sqrt(DK)
    wq16 = (Wq * scale).astype(np.float16)
    bq_s = (bq * scale).astype(np.float32)

    x_min, y_min, x_max, y_max = [box[..., i] for i in range(4)]
    cx = (x_min + x_max) * 0.5
    cy = (y_min + y_max) * 0.5
    w = x_max - x_min + 1.0
    hh = y_max - y_min + 1.0
    nsd = np.stack([cx, cy, 1e-3 * w, 1e-3 * hh,
                    np.log(w), np.log(hh)], axis=1).astype(np.float32)

    # m-side values permuted by PSI within each 128-block
    perm = np.concatenate([np.array(PSI), 128 + np.array(PSI)])
    msd = np.stack([cx[:, perm], cy[:, perm]], axis=1).astype(np.float32)
    mkb = ((mask.astype(np.float32) - 1.0) * 1e9)[:, perm].astype(np.float32)

    # contraction weights: wpc[comp*2+fh] [128 rows=(m'*8+f_loc), 128 cols=(h*16+m')]
    wpc = np.zeros((4, 128, 128), dtype=np.float32)
    for comp in range(2):
        for fh in range(2):
            for mp in range(16):
                for fl in range(8):
                    f = fh * 8 + fl          # f = j*2 + trig
                    j, trig = divmod(f, 2)
                    val_idx = (32 * trig) + comp * 8 + j
                    for h in range(H):
                        wpc[comp * 2 + fh, mp * 8 + fl, h * 16 + mp] = \
                            WG_w[h, val_idx]
    wpc = wpc.astype(np.float16)

    mixh = np.zeros((H, 33, 33), dtype=np.float32)
    for h in range(H):
        for i in (2, 3):
            for j in range(8):
                rb_s = (i - 2) * 16 + j * 2 + 0
                rb_c = rb_s + 1
                ws = WG_w[h, i * 8 + j]
                wc = WG_w[h, 32 + i * 8 + j]
                rc0 = (i - 2) * 16 + j * 2 + 0
                rc1 = rc0 + 1
                mixh[h, rb_c, rc0] = ws
                mixh[h, rb_s, rc0] = wc
                mixh[h, rb_c, rc1] = wc
                mixh[h, rb_s, rc1] = -ws
        mixh[h, 32, 32] = WG_b[h]
    mixh = mixh.astype(np.float16)

    fsc = np.zeros((33, 2), dtype=np.float32)
    for i in (2, 3):
        for j in range(8):
            for trig in range(2):
                r = (i - 2) * 16 + j * 2 + trig
                fsc[r, 0] = CJ[j] * 65536.0 / TWO_PI
                fsc[r, 1] = (trig * 0.25 + 0.5 + SBIG) * 65536.0
    fsc[32, 1] = (0.5 + SBIG) * 65536.0

    shared = dict(wq=wq16, wk=Wk.astype(np.float16), wv=Wv.astype(np.float16),
                  wo=Wo.astype(np.float16), bqc=bq_s, bkc=bk, bvc=bv, boc=bo,
                  wpc=wpc, mixh=mixh, fsc=fsc,
                  iden=np.eye(128, dtype=np.float32))
    in_maps = []
    for c in range(NCORES):
        sl = slice(c * BL, (c + 1) * BL)
        m = dict(shared)
        m.update(xq=iq[sl], xk=ik[sl], xv=iv[sl], nsd=nsd[sl], msd=msd[sl],
                 mkb=mkb[sl])
        in_maps.append(m)
    return in_maps


def kernel(**inputs):
    from concourse.bass_utils import run_bass_kernel_spmd

    if "nc" not in _CACHE:
        nc = _build_nc()
        nc.finalize()
        _CACHE["nc"] = nc
    nc = _CACHE["nc"]

    in_maps = _host_prep(inputs)
    res = run_bass_kernel_spmd(nc, in_maps, list(range(NCORES)))
    outs = [res.results[c]["out"] for c in range(NCORES)]
    return np.concatenate(outs, axis=0).astype(np.float32)


if __name__ == "__main__":
    nc = _build_nc()
    nc.finalize()
    print("build ok")
